# revision 10
# baseline (speedup 1.0000x reference)
"""BertAttention (B=2, S=2048, D=1024, H=16) on 8 trn2 NeuronCores.

Sharding: core c handles batch b = c // 4 and query-row slice r = c % 4
(rows 512r .. 512r+512 of that batch). Each core computes K/V projections
for its *entire* batch (4x duplicated inside a batch group - this avoids
any cross-core collective), and Q / attention / Wo / LayerNorm only for
its own 512 rows. The host pre-transposes hidden states to [D, S] layout
and rotates the sequence so every core's own rows sit at columns 0..511;
the SPMD program is then identical on all 8 cores.

Math folds (exact):
 - scores scale 1/sqrt(64) folded into Wq/bq on host
 - bk dropped entirely: softmax(q.(k+bk)) == softmax(q.k) (shift invariance)
 - bv folded into bo on host: bo' = bo + bv @ Wo
 - softmax denominators come from an extra ones-column appended to V, so
   the PE produces sum_t exp(s) alongside ctx; the divide is applied to
   ctx (per head) before the Wo matmul, using a K=1 ones-matmul to
   broadcast 1/denom across partitions.
Matmuls run in float32r (TF32-like, 1 cycle/row at N=512; plain fp32 is
4x slower).
"""

import sys

sys.path.insert(0, "/opt/trn_rl_repo")
import numpy as np

B, S, D = 2, 2048, 1024
H, DH = 16, 64
N_CORES = 8
SQ = 512           # own rows per core == t-quarter size
NQ = 4             # t quarters per batch
KC = 8             # 128-row contraction chunks of D
LN_EPS = 1e-12

_CACHE = {}


def _build():
    import concourse.bass as bass
    from concourse import bacc, mybir
    import concourse.tile as tile

    F32 = mybir.dt.float32
    F32R = mybir.dt.float32r
    ALU = mybir.AluOpType
    ACTF = mybir.ActivationFunctionType

    nc = bacc.Bacc("TRN2", target_bir_lowering=False, debug=False,
                   num_devices=N_CORES)

    xT = nc.dram_tensor("xT", [D, S], F32R, kind="ExternalInput").ap()
    wq = nc.dram_tensor("wq", [D, D], F32R, kind="ExternalInput").ap()
    wk = nc.dram_tensor("wk", [D, D], F32R, kind="ExternalInput").ap()
    wv = nc.dram_tensor("wv", [D, D], F32R, kind="ExternalInput").ap()
    wo = nc.dram_tensor("wo", [D, D], F32R, kind="ExternalInput").ap()
    bq_c = nc.dram_tensor("bq_c", [128, KC], F32, kind="ExternalInput").ap()
    xbo = nc.dram_tensor("xbo", [SQ, D], F32, kind="ExternalInput").ap()
    gam = nc.dram_tensor("gam", [128, D], F32, kind="ExternalInput").ap()
    onesc = nc.dram_tensor("onesc", [128, 64], F32R, kind="ExternalInput").ap()
    bet = nc.dram_tensor("bet", [128, D], F32, kind="ExternalInput").ap()
    out = nc.dram_tensor("out", [SQ, D], F32, kind="ExternalOutput").ap()

    with tile.TileContext(nc) as tc:
        with (
            tc.tile_pool(name="persist", bufs=1) as pp,
            tc.tile_pool(name="xtq", bufs=8) as xpool,
            tc.tile_pool(name="ktp", bufs=9) as kpool,
            tc.tile_pool(name="vp", bufs=5) as vpool,
            tc.tile_pool(name="wch", bufs=4) as wpool,
            tc.tile_pool(name="expp", bufs=3) as epool,
            tc.tile_pool(name="epi", bufs=2) as hpool,
            tc.tile_pool(name="rcp", bufs=2) as rpool,
            tc.tile_pool(name="ps_proj", bufs=4, space="PSUM") as ps_proj,
            tc.tile_pool(name="ps_sc", bufs=2, space="PSUM") as ps_sc,
            tc.tile_pool(name="ps_ctx", bufs=2, space="PSUM") as ps_ctx,
        ):
            # ---- persistent tiles ----
            qT = pp.tile([128, KC, SQ], F32R, name="qT")
            ctx = pp.tile([128, KC, SQ], F32R, name="ctx")
            denom = pp.tile([1, H, SQ], F32, name="denom")
            gam_sb = pp.tile([128, D], F32, name="gam_sb")
            bet_sb = pp.tile([128, D], F32, name="bet_sb")
            bq_sb = pp.tile([128, KC], F32, name="bq_sb")
            ones_r = pp.tile([1, 64], F32R, name="ones_r")
            ones_f = pp.tile([128, 16], F32, name="ones_f")
            eps_sb = pp.tile([128, 1], F32, name="eps_sb")

            nc.sync.dma_start(gam_sb, gam)
            nc.sync.dma_start(bet_sb, bet)
            nc.sync.dma_start(bq_sb, bq_c)
            nc.sync.dma_start(ones_r, onesc[0:1, :])
            nc.vector.memset(ones_f, 1.0)
            nc.vector.memset(eps_sb, LN_EPS)

            for q in range(NQ):
                # ---- xT quarter chunks ----
                xtiles = []
                for kc in range(KC):
                    xt = xpool.tile([128, 512], F32R,
                                    name=f"xt_{q}_{kc}", tag="xt")
                    nc.sync.dma_start(
                        xt, xT[kc * 128:(kc + 1) * 128,
                               q * 512:(q + 1) * 512])
                    xtiles.append(xt)

                if q == 0:
                    # ---- Q projection (own rows only), two dk halves ----
                    for half in range(2):
                        qps = [ps_proj.tile([128, 512], F32,
                                            name=f"qps{half}_{j}", tag="proj")
                               for j in range(4)]
                        col = slice(half * 512, (half + 1) * 512)
                        for kc in range(KC):
                            wt = wpool.tile([128, 512], F32R,
                                            name=f"wq_{half}_{kc}", tag="wch")
                            nc.sync.dma_start(
                                wt, wq[kc * 128:(kc + 1) * 128, col])
                            for j in range(4):
                                nc.tensor.matmul(
                                    qps[j], wt[:, j * 128:(j + 1) * 128],
                                    xtiles[kc],
                                    start=(kc == 0), stop=(kc == KC - 1))
                        for j in range(4):
                            dk = half * 4 + j
                            nc.vector.tensor_scalar_add(
                                qT[:, dk], qps[j], bq_sb[:, dk:dk + 1])

                # ---- K^T projection: out kT[dk, t], two dk halves ----
                ktiles = []
                for half in range(2):
                    kps = [ps_proj.tile([128, 512], F32,
                                        name=f"kps_{q}_{half}_{j}", tag="proj")
                           for j in range(4)]
                    col = slice(half * 512, (half + 1) * 512)
                    for kc in range(KC):
                        wt = wpool.tile([128, 512], F32R,
                                        name=f"wk_{q}_{half}_{kc}", tag="wch")
                        nc.sync.dma_start(
                            wt, wk[kc * 128:(kc + 1) * 128, col])
                        for j in range(4):
                            nc.tensor.matmul(
                                kps[j], wt[:, j * 128:(j + 1) * 128],
                                xtiles[kc],
                                start=(kc == 0), stop=(kc == KC - 1))
                    for j in range(4):
                        kt = kpool.tile([128, 512], F32R,
                                        name=f"kt_{q}_{half}_{j}", tag="kt")
                        nc.vector.tensor_copy(kt, kps[j])
                        ktiles.append(kt)

                # ---- V projection: out v[t, dv] packed per head with a
                # ones column: v tile [128, 16*65]; head h cols 65h..65h+63,
                # ones at 65h+64 ----
                vtiles = []
                for tt in range(4):
                    vt = vpool.tile([128, H * 65], F32R,
                                    name=f"v_{q}_{tt}", tag="v")
                    vtiles.append(vt)
                    nc.vector.tensor_copy(
                        vt.rearrange("p (h c) -> p h c", c=65)[:, :, 64:65],
                        ones_f.rearrange("p (a b) -> p a b", b=1))
                for half in range(2):
                    vps = [ps_proj.tile([128, 512], F32,
                                        name=f"vps_{q}_{half}_{j}", tag="proj")
                           for j in range(4)]
                    col = slice(half * 512, (half + 1) * 512)
                    for kc in range(KC):
                        wt = wpool.tile([128, 512], F32R,
                                        name=f"wv_{q}_{half}_{kc}", tag="wch")
                        nc.sync.dma_start(
                            wt, wv[kc * 128:(kc + 1) * 128, col])
                        for tt in range(4):
                            nc.tensor.matmul(
                                vps[tt],
                                xtiles[kc][:, tt * 128:(tt + 1) * 128], wt,
                                start=(kc == 0), stop=(kc == KC - 1))
                    for tt in range(4):
                        dst = vtiles[tt].rearrange(
                            "p (h c) -> p h c",
                            c=65)[:, half * 8:(half + 1) * 8, 0:64]
                        src = vps[tt].rearrange("p (h c) -> p h c", c=64)
                        nc.vector.tensor_copy(dst, src)

                # ---- attention for this quarter ----
                for h in range(H):
                    dkc, poff = h // 2, (h % 2) * 64
                    cps = ps_ctx.tile([65, 512], F32,
                                      name=f"ctxps_{q}_{h}", tag="ctx")
                    for tc_ in range(4):
                        sps = ps_sc.tile([128, 512], F32,
                                         name=f"scps_{q}_{h}_{tc_}", tag="sc")
                        nc.tensor.matmul(
                            sps,
                            ktiles[dkc][poff:poff + 64,
                                        tc_ * 128:(tc_ + 1) * 128],
                            qT[poff:poff + 64, dkc],
                            start=True, stop=True)
                        et = epool.tile([128, 512], F32R,
                                        name=f"exp_{q}_{h}_{tc_}", tag="exp")
                        nc.scalar.activation(et, sps, ACTF.Exp)
                        nc.tensor.matmul(
                            cps, vtiles[tc_][:, 65 * h:65 * h + 65], et,
                            start=(tc_ == 0), stop=(tc_ == 3))
                    # evict ctx rows + denom row, accumulating over quarters
                    if q == 0:
                        nc.vector.tensor_copy(ctx[poff:poff + 64, dkc],
                                              cps[0:64])
                        nc.vector.tensor_copy(denom[:, h], cps[64:65])
                    else:
                        nc.vector.tensor_tensor(
                            ctx[poff:poff + 64, dkc],
                            cps[0:64], ctx[poff:poff + 64, dkc], ALU.add)
                        dtmp = hpool.tile([1, 512], F32,
                                          name=f"dtmp_{q}_{h}", tag="dtmp")
                        nc.vector.tensor_copy(dtmp, cps[64:65])
                        nc.vector.tensor_tensor(
                            denom[:, h], dtmp, denom[:, h], ALU.add)

            # ---- normalize ctx by softmax denominators (per head) ----
            for h in range(H):
                dkc, poff = h // 2, (h % 2) * 64
                rch = rpool.tile([1, SQ], F32R, name=f"rcp_{h}", tag="rcp")
                with nc.allow_low_precision(reason="f32r recip for bcast mm"):
                    nc.vector.reciprocal(rch, denom[:, h])
                rb = ps_ctx.tile([64, 512], F32, name=f"rb_{h}", tag="ctx")
                nc.tensor.matmul(rb, ones_r, rch, start=True,
                                 stop=True)
                nc.vector.tensor_tensor(
                    ctx[poff:poff + 64, dkc],
                    ctx[poff:poff + 64, dkc], rb, ALU.mult)

            # ---- Wo matmul + residual + LayerNorm per own s-tile ----
            h_tiles = [hpool.tile([128, D], F32, name=f"h_{st}", tag="h",
                                  bufs=4) for st in range(4)]
            for half in range(2):
                col = slice(half * 512, (half + 1) * 512)
                ops_ = [ps_proj.tile([128, 512], F32,
                                     name=f"ho_{half}_{st}", tag="proj")
                        for st in range(4)]
                for kc in range(KC):
                    wt = wpool.tile([128, 512], F32R,
                                    name=f"wo_{half}_{kc}", tag="wch")
                    nc.sync.dma_start(wt, wo[kc * 128:(kc + 1) * 128, col])
                    for st in range(4):
                        nc.tensor.matmul(
                            ops_[st], ctx[:, kc, st * 128:(st + 1) * 128],
                            wt, start=(kc == 0), stop=(kc == KC - 1))
                for st in range(4):
                    nc.vector.tensor_copy(h_tiles[st][:, col], ops_[st])

            for st in range(4):
                xb = hpool.tile([128, D], F32, name=f"xb_{st}", tag="xb",
                                bufs=2)
                nc.sync.dma_start(xb, xbo[st * 128:(st + 1) * 128, :])
                h_sb = h_tiles[st]
                nc.vector.tensor_tensor(h_sb, h_sb, xb, ALU.add)
                mu = hpool.tile([128, 1], F32, name=f"mu_{st}", tag="mu")
                nc.vector.reduce_sum(mu, h_sb, axis=mybir.AxisListType.X)
                nc.vector.tensor_scalar_mul(mu, mu, 1.0 / D)
                hc = hpool.tile([128, D], F32, name=f"hc_{st}", tag="hc")
                nc.vector.tensor_scalar_sub(hc, h_sb, mu)
                sq = hpool.tile([128, D], F32, name=f"sq_{st}", tag="xb",
                                bufs=2)
                var = hpool.tile([128, 1], F32, name=f"var_{st}", tag="var")
                nc.vector.tensor_tensor(sq, hc, hc, ALU.mult)
                nc.vector.reduce_sum(var, sq, axis=mybir.AxisListType.X)
                nc.vector.tensor_scalar_mul(var, var, 1.0 / D)
                sd = hpool.tile([128, 1], F32, name=f"sd_{st}", tag="sd")
                nc.scalar.activation(sd, var, ACTF.Sqrt, bias=eps_sb,
                                     scale=1.0)
                rs = hpool.tile([128, 1], F32, name=f"rs_{st}", tag="rs")
                nc.vector.reciprocal(rs, sd)
                o1 = hpool.tile([128, D], F32, name=f"o1_{st}", tag="h",
                                bufs=4)
                nc.vector.scalar_tensor_tensor(
                    o1, hc, rs, gam_sb, ALU.mult, ALU.mult)
                o2 = hpool.tile([128, D], F32, name=f"o2_{st}", tag="hc")
                nc.vector.tensor_tensor(o2, o1, bet_sb, ALU.add)
                nc.sync.dma_start(out[st * 128:(st + 1) * 128, :], o2)

    nc.compile()
    return nc


def _prep_inputs(hidden_states, Wq, bq, Wk, bk, Wv, bv, Wo, bo,
                 ln_gamma, ln_beta):
    f = np.float32
    hidden = np.asarray(hidden_states, f)
    Wq = np.asarray(Wq, f) * np.float32(1.0 / np.sqrt(DH))
    bq = np.asarray(bq, f) * np.float32(1.0 / np.sqrt(DH))
    Wo = np.asarray(Wo, f)
    bo_eff = (np.asarray(bo, f) + np.asarray(bv, f) @ Wo).astype(f)
    gam_b = np.ascontiguousarray(
        np.broadcast_to(np.asarray(ln_gamma, f), (128, D)))
    bet_b = np.ascontiguousarray(
        np.broadcast_to(np.asarray(ln_beta, f), (128, D)))
    bq_c = np.ascontiguousarray(bq.reshape(KC, 128).T)

    in_maps = []
    for c in range(N_CORES):
        b, r = c // NQ, c % NQ
        xb = hidden[b]                                   # [S, D]
        xrot = np.roll(xb, -SQ * r, axis=0)
        in_maps.append({
            "xT": np.ascontiguousarray(xrot.T),          # [D, S]
            "wq": Wq, "wk": np.asarray(Wk, f),
            "wv": np.asarray(Wv, f), "wo": Wo,
            "bq_c": bq_c,
            "xbo": (xb[SQ * r:SQ * (r + 1)] + bo_eff).astype(f),
            "gam": gam_b, "bet": bet_b,
            "onesc": np.ones((128, 64), np.float32),
        })
    return in_maps


def kernel(hidden_states, Wq, bq, Wk, bk, Wv, bv, Wo, bo,
           ln_gamma, ln_beta):
    from concourse.bass_utils import run_bass_kernel_spmd

    if "nc" not in _CACHE:
        _CACHE["nc"] = _build()
    nc = _CACHE["nc"]

    in_maps = _prep_inputs(hidden_states, Wq, bq, Wk, bk, Wv, bv, Wo, bo,
                           ln_gamma, ln_beta)
    res = run_bass_kernel_spmd(nc, in_maps, core_ids=list(range(N_CORES)))

    out = np.empty((B, S, D), np.float32)
    for c in range(N_CORES):
        b, r = c // NQ, c % NQ
        out[b, SQ * r:SQ * (r + 1)] = res.results[c]["out"]
    return out
